# revision 29
# baseline (speedup 1.0000x reference)
"""Trainium2 Bass kernel for nn_Map_79748952752358 (dense_cnn).

Pipeline: LSTM encoder (40 steps) -> e2d projection -> big linear
(lin1: 256 -> 262144) -> per-sample dynamic 1x1 conv over feature
[1024, 32x32] -> BN(eval) -> channel-max -> clip.

Sharding (v3):
  - LSTM + e2d replicated on all 8 cores (serial recurrence, tiny state).
  - lin1 tensor-sharded over C: core k computes f1[all r, c-chunk k] for
    all 16 samples (W1 shard streams in fp8, 8.4MB/core, fully resident
    in SBUF before lin1 starts).
  - Two AllToAlls (one per r-half) redistribute filters so core k holds
    the full [256, 1024] filter block for ITS two samples; payload fp8.
  - conv batch-sharded: each core convolves its 2 samples with full
    R=256; BN shift enters via the evacuation ACT bias.
  - per-core output: channel-max partials [128, 128]; host combines the
    32-row groups, applies the BN-shift floor T0 and the clip.

Quantization (exact power-of-2 scale folds, verified vs fp32 reference
host-side: l2_rel ~5.3e-3 < 2e-2 gate):
  - W1 stored fp8e4 as 512*W1'; bias inject 512*b1; evacuation computes
    f1q = prelu(psum * 0.5) = 256*lrelu(W1'x+b1) stored fp8e4.
  - conv PSUM = 256*rele; evacuation ACT scale 1/256 + BN-shift bias.
  - W_hh stored fp8e4 as 256*w (g-rows pre-doubled for the
    tanh(g)=2*sig(2g)-1 trick); w_ih/bias scaled 256 in bf16/f32 so the
    gate PSUM is 256*gates; sigmoid ACT scale 1/256 unscales exactly.
"""

import numpy as np
import ml_dtypes

import concourse.bass as bass
import concourse.mybir as mybir
from concourse import tile
from concourse.tile import ScopedClock
from concourse.alu_op_type import AluOpType
from concourse.bass_utils import run_bass_kernel_spmd

BF16 = ml_dtypes.bfloat16
FP8 = ml_dtypes.float8_e4m3

B, S, V, E, HID = 16, 40, 1004, 256, 256
C, R, HW2 = 1024, 256, 1024
BN_EPS = 1e-5
N_CORES = 8
RS = R // N_CORES   # 32 r-rows per lin1 psum tile
BS = B // N_CORES   # 2 samples per core (conv shard)
P = 128

SC_HH = 256.0   # gate-path scale (w_ih, w_hh, gate bias, xg)
SC_W1 = 512.0   # W1 fp8 scale
SC_F1 = 256.0   # f1 fp8 scale

AFT = mybir.ActivationFunctionType
AX = mybir.AxisListType


# ---------------------------------------------------------------------------
# Tile tail-drain patch: this walrus build accepts fewer sem waits per
# TPB_CTRL instruction than Tile's exit drain accumulates; split them into
# single-wait SP nops.
_drain_patched = False


def _patch_tile_drain():
    global _drain_patched
    if _drain_patched:
        return
    _drain_patched = True

    def _patched(self, tick_clock, wait_clock):
        nc = self.nc
        probe = nc.sync.nop(nofuse=True, hint="drain_wait_split")
        wait_clock.add_sem_waits(
            probe.ins, ScopedClock({None: tick_clock.global_clock})
        )
        si = probe.ins.sync_info
        waits = list(si.on_wait or []) if si is not None else []
        if len(waits) > 1:
            si.on_wait = waits[:1]
            for w in waits[1:]:
                n = nc.sync.nop(nofuse=True, hint="drain_wait_split")
                nsi = n.ins.sync_info
                if nsi is None:
                    import bass_rust

                    n.ins.sync_info = bass_rust.SyncInfo(on_wait=[w], on_update=[])
                else:
                    nsi.on_wait = [w]
        nc.sync.drain()
        nc.all_engine_barrier()
        assert self.sems is not None
        popped = nc._tile_sem_poison_stack.pop()
        assert popped is self._sem_poison
        nc.clear_and_free_semaphores(list(self.sems.allocated().values()))
        nc.all_engine_barrier()

    tile.TileContext._drain_and_barrier = _patched


_ws_counter = [0]


def _split_excess_waits(nc, limit=1):
    """Walrus on this image rejects instructions with more than ~2 sem waits.
    Move excess waits onto same-engine EventSemaphore carriers inserted just
    before the offending instruction (same per-engine stream order, identical
    blocking semantics)."""
    import bass_rust

    for fn in nc.m.functions:
        for bb in fn.blocks:
            out = []
            for inst in bb.instructions:
                si = inst.sync_info
                waits = list(si.on_wait or []) if si is not None else []
                if len(waits) > limit:
                    for w in waits[:-limit]:
                        _ws_counter[0] += 1
                        carrier = mybir.InstEventSemaphore(
                            name=f"I-waitsplit-{_ws_counter[0]}",
                            opcode="EventSemaphore",
                            engine=inst.engine,
                            sync_info=bass_rust.SyncInfo(
                                on_wait=[w], on_update=[]),
                        )
                        out.append(carrier)
                    si.on_wait = waits[-limit:]
                out.append(inst)
            bb.instructions = out


# ---------------------------------------------------------------------------
def _build_program(slots):
    """Build the SPMD Bass program. `slots[b]` = length[b]-1, the LSTM step
    whose hidden state is each sample's final state (compile-time constants).
    """
    _patch_tile_drain()
    nc = bass.Bass("TRN2", target_bir_lowering=False, debug=False,
                   num_devices=N_CORES)
    dt = mybir.dt
    f32, bf16, fp8 = dt.float32, dt.bfloat16, dt.float8e4

    def din(name, shape, d=bf16):
        return nc.dram_tensor(name, shape, d, kind="ExternalInput").ap()

    # feature slice for this core's 2 samples: (b, c_in_chunk, kc, hw)
    feat_d = din("feat", [BS, P, 8, HW2])
    # packed bf16 constant block: embT | wihT | e2dT | eye | b1 | delta |
    # e2dbias [2,128] | e2dsel [2,32]
    PK = [2 * S * B, 16 * P, 4 * P, P, 8 * P, RS * B, P + 2 * B]
    pack_d = din("cpack", [P, sum(PK)])
    whh_d = din("whh8", [P, 16 * P], fp8)
    biasf_d = din("biasf", [P, 12], f32)   # biasg(8)*256 | e2db(2) | tvec(2)
    w1_d = din("w1T", [P, 8 * RS * 2 * P], fp8)   # tiles (rt, r, kh)

    out_d = nc.dram_tensor("part_out", [P, BS * 2 * 2 * 16], f32,
                           kind="ExternalOutput").ap()

    with tile.TileContext(nc) as tc:
        with (
            tc.tile_pool(name="const", bufs=1) as cpool,
            tc.tile_pool(name="xg", bufs=1) as xgpool,
            tc.tile_pool(name="hist", bufs=1) as hpool,
            tc.tile_pool(name="gs", bufs=2) as gspool,
            tc.tile_pool(name="cell", bufs=1) as cellpool,
            tc.tile_pool(name="tmp", bufs=4) as tmppool,
            tc.tile_pool(name="w1c", bufs=8) as w1pool,
            tc.tile_pool(name="f1", bufs=1) as f1pool,
            tc.tile_pool(name="feat", bufs=2) as fpool,
            tc.tile_pool(name="f1c", bufs=2) as f1cpool,
            tc.tile_pool(name="rmax", bufs=4) as rmpool,
            tc.tile_pool(name="vout", bufs=1) as vpool,
            tc.tile_pool(name="a2a", bufs=1, space="DRAM") as dpool,
        ):
            pack = cpool.tile([P, sum(PK)], bf16, tag="cpack")
            # split the pack load: embT+wih first so stage A starts early
            cut = PK[0] + PK[1]
            nc.sync.dma_start(out=pack[:, 0:cut], in_=pack_d[:, 0:cut])
            biasf = cpool.tile([P, 12], f32, tag="biasf")
            nc.sync.dma_start(out=biasf[:], in_=biasf_d)
            whh = cpool.tile([P, 16 * P], fp8, tag="whh8")
            nc.sync.dma_start(out=whh[:], in_=whh_d)
            nc.sync.dma_start(out=pack[:, cut:], in_=pack_d[:, cut:])

            # prepull the sigmoid/tanh ACT table set with a dummy activation
            # so the one-time ~1.3us table load overlaps the pack DMA
            # instead of delaying LSTM step 0.
            warm = cpool.tile([1, 1], f32, tag="warm")
            nc.scalar.activation(out=warm[:], in_=biasf[0:1, 0:1],
                                 func=AFT.Sigmoid)

            # early cross-core barrier: aligns all cores while the LSTM
            # runs (gpsimd/ncfw idle), so the real AllToAll's entry skew
            # shrinks.  Costs nothing on the critical path.
            bar_sb = cpool.tile([1, 8], f32, tag="bar")
            nc.vector.memset(bar_sb[:], 1.0)
            bar_in = dpool.tile([1, 8], f32, name="barin")
            bar_out = dpool.tile([1, 8], f32, name="barout")
            nc.gpsimd.dma_start(out=bar_in[:], in_=bar_sb[:])
            nc.gpsimd.collective_compute(
                "AllReduce", AluOpType.add,
                replica_groups=[list(range(N_CORES))],
                ins=[bar_in[:].opt()], outs=[bar_out[:].opt()],
            )

            off = np.cumsum([0] + PK)
            embT = pack[:, off[0]:off[1]]
            wih = pack[:, off[1]:off[2]]
            e2dT = pack[:, off[2]:off[3]]
            eye = pack[:, off[3]:off[4]]
            b1 = pack[0:RS, off[4]:off[5]]
            delta = pack[0:RS, off[5]:off[6]]
            e2dbias = pack[0:2, off[6]:off[6] + P]        # [2, 128]
            e2dsel = pack[0:2, off[6] + P:off[7]]         # [2, 32]
            biasg = biasf[:, 0:8]
            tvec = biasf[:, 10:12]                # BN shift per conv r-tile

            # ---- prefetched loads: all 8 w1 chunks (fp8), feature --------
            CW = RS * 2 * P  # w1 chunk cols per ct

            def w1_fetch(ct):
                wch = w1pool.tile([P, CW], fp8, tag="w1c", name=f"wch{ct}")
                nc.sync.dma_start(out=wch[:], in_=w1_d[:, ct * CW:(ct + 1) * CW])
                return wch

            wchs = []
            for ct in range(8):
                wchs.append(w1_fetch(ct))
                if ct == 3:
                    # feature slice DMA sits between w1 chunk 3 and 4 so it
                    # is in flight well before the conv needs it.
                    fbs = []
                    for b in range(BS):
                        fb = fpool.tile([P, 8 * HW2], bf16, tag="feat",
                                        name=f"fb{b}")
                        nc.sync.dma_start(
                            out=fb[:].rearrange("p (kc hw) -> p kc hw", kc=8),
                            in_=feat_d[b])
                        fbs.append(fb)

            # ---- Stage A: xg = 256*(w_ih @ x_t + bg) for all steps -------
            # n=0 (steps 0-19) runs before the LSTM; n=1 (steps 20-39) is
            # emitted inside LSTM steps 0-7 to fill engine idle slots.
            xg_s = xgpool.tile([P, 8 * S * B], bf16)
            NCH = 320  # psum N-chunk: 20 steps x 16
            lstm_psum = tc.tile_pool(name="gpsum", bufs=2, space="PSUM")
            gpsum = lstm_psum.__enter__()
            xpsum_cm = tc.tile_pool(name="xpsum", bufs=2, space="PSUM")
            xpsum = xpsum_cm.__enter__()

            def stage_a(n, m):
                ps = xpsum.tile([P, NCH], f32, tag="xg")
                for ke in range(2):
                    nc.tensor.matmul(
                        ps[:],
                        lhsT=wih[:, (ke * 8 + m) * P:(ke * 8 + m + 1) * P],
                        rhs=embT[:, ke * S * B + n * NCH: ke * S * B + (n + 1) * NCH],
                        start=(ke == 0), stop=(ke == 1),
                    )
                dst = xg_s[:, m * S * B + n * NCH: m * S * B + (n + 1) * NCH]
                if m % 2 == 0:
                    nc.scalar.activation(out=dst, in_=ps[:], func=AFT.Identity,
                                         bias=biasg[:, m:m + 1])
                else:
                    nc.vector.tensor_scalar_add(dst, ps[:], biasg[:, m:m + 1])

            for m in range(8):
                stage_a(0, m)

            # ---- Stage B: LSTM recurrence (layout: gate-dim on partitions) --
            # gates psum split: ifg (cols 0:96) and o (32) so the sigmoid of
            # i,f,g starts after 12 of the 16 whh matmuls and sig_o overlaps
            # the DVE c-update chain.
            S_eff = max(slots) + 1
            hist = hpool.tile([P, S * 2 * B], bf16)   # (t, kh, b)
            c_s = cellpool.tile([P, 2 * B], f32)      # (kh, b)
            xg_r = xg_s[:].rearrange("p (m t b) -> p m t b", m=8, t=S)
            # final-h capture target (filled inside the loop at t==slots[b])
            h_fin = cellpool.tile([P, 2 * B], bf16, tag="hfin")  # (kh, b)
            hf_r = h_fin[:].rearrange("p (k b) -> p b k", k=2)
            inv_hh = 1.0 / SC_HH

            def whh_mm(dst, dcol, m, kh, t, last):
                nc.tensor.matmul(
                    dst[:, dcol * B:(dcol + 1) * B],
                    lhsT=whh[:, (kh * 8 + m) * P:(kh * 8 + m + 1) * P],
                    rhs=hist[:, (t - 1) * 2 * B + kh * B:
                             (t - 1) * 2 * B + (kh + 1) * B],
                    start=False, stop=last, skip_group_check=True,
                )

            for t in range(S_eff):
                # psum/ACT split (g), (if), (o): sig_g lands first so the
                # tg tensor_scalar overlaps sig_if; sig_o overlaps the DVE
                # c-chain.  Gate tiles: i=m0-1, f=m2-3, g=m4-5, o=m6-7.
                # separate psum TILES per gate group: Tile tracks psum reads
                # at tile granularity, so sig_g can fire right after the 4
                # g-gate matmuls instead of all 16.  Separate tiles also sit
                # in separate banks, keeping one start=True per bank.
                gg_t = gpsum.tile([P, 32], f32, tag="gg")
                gif_t = gpsum.tile([P, 64], f32, tag="gif")
                go_t = gpsum.tile([P, 32], f32, tag="go")
                gg, gif, go = gg_t[:], gif_t[:], go_t[:]
                nc.tensor.matmul(gg, lhsT=eye[:], rhs=xg_r[:, 4:6, t, :],
                                 start=True, stop=(t == 0),
                                 skip_group_check=True)
                nc.tensor.matmul(gif, lhsT=eye[:], rhs=xg_r[:, 0:4, t, :],
                                 start=True, stop=(t == 0),
                                 skip_group_check=True)
                nc.tensor.matmul(go, lhsT=eye[:], rhs=xg_r[:, 6:8, t, :],
                                 start=True, stop=(t == 0),
                                 skip_group_check=True)
                if t > 0:
                    for m in (4, 5):
                        for kh in range(2):
                            whh_mm(gg, m - 4, m, kh, t, m == 5 and kh == 1)
                    for m in range(4):
                        for kh in range(2):
                            whh_mm(gif, m, m, kh, t, m == 3 and kh == 1)
                    for m in (6, 7):
                        for kh in range(2):
                            whh_mm(go, m - 6, m, kh, t, m == 7 and kh == 1)
                gsg = gspool.tile([P, 32], f32, tag="gsg")
                gsif = gspool.tile([P, 64], f32, tag="gsif")
                gso = gspool.tile([P, 32], f32, tag="gso")
                # tanh(g)=2*sig(2g)-1 (g-rows pre-scaled by 2 on host)
                nc.scalar.activation(out=gsg[:], in_=gg, func=AFT.Sigmoid,
                                     scale=inv_hh)
                nc.scalar.activation(out=gsif[:], in_=gif, func=AFT.Sigmoid,
                                     scale=inv_hh)
                nc.scalar.activation(out=gso[:], in_=go, func=AFT.Sigmoid,
                                     scale=inv_hh)
                # half-scale cell state: c' = c/2, so i*tanh(g) becomes one
                # fused op (sig_g - 0.5)*sig_i; tanh reads with scale=2.
                t1 = tmppool.tile([P, 2 * B], f32, tag="t1")
                if t == 0:
                    nc.vector.scalar_tensor_tensor(
                        t1[:], gsg[:], 0.5, gsif[:, 0:32],
                        AluOpType.subtract, AluOpType.mult)
                    nc.vector.tensor_copy(c_s[:], t1[:])
                else:
                    t2 = tmppool.tile([P, 2 * B], f32, tag="t2")
                    nc.vector.tensor_tensor(t2[:], gsif[:, 32:64], c_s[:],
                                            AluOpType.mult)
                    nc.vector.scalar_tensor_tensor(
                        t1[:], gsg[:], 0.5, gsif[:, 0:32],
                        AluOpType.subtract, AluOpType.mult)
                    nc.vector.tensor_tensor(c_s[:], t1[:], t2[:], AluOpType.add)
                th = tmppool.tile([P, 2 * B], bf16, tag="th")
                nc.scalar.activation(out=th[:], in_=c_s[:], func=AFT.Tanh,
                                     scale=2.0)
                nc.vector.tensor_tensor(
                    hist[:, t * 2 * B:(t + 1) * 2 * B],
                    gso[:], th[:], AluOpType.mult)
                # capture final h for samples ending at this step (hides in
                # the per-step DVE slack)
                src = hist[:, t * 2 * B:(t + 1) * 2 * B]
                for b in range(B):
                    if slots[b] == t:
                        nc.vector.tensor_copy(
                            hf_r[:, b],
                            src.rearrange("p (k b) -> p b k", k=2)[:, b])
                # stage A n=1: fills the PE/ACT/DVE idle slots of the first
                # 8 steps (needed from step 20 onward only)
                if t < 8:
                    stage_a(1, t)
            xpsum_cm.__exit__(None, None, None)

            # ---- e2d projection: instrT = tanh(e2d_w @ h + b) -------------
            # one psum group; bias injected via rank-1 matmuls so a single
            # tanh ACT covers both m-tiles.
            instrT = cellpool.tile([P, 2 * B], bf16, tag="instrT")  # (kh, b)
            pe2 = gpsum.tile([P, 2 * B], f32, tag="gg")  # reuse gate bank
            # rank-2 bias inject: one start=True matmul covers both m-tiles
            nc.tensor.matmul(pe2[:], lhsT=e2dbias[0:2, 0:P],
                             rhs=e2dsel[0:2, :],
                             start=True, stop=False, skip_group_check=True)
            for m in range(2):
                for kh in range(2):
                    nc.tensor.matmul(
                        pe2[:, m * B:(m + 1) * B],
                        lhsT=e2dT[:, (kh * 2 + m) * P:(kh * 2 + m + 1) * P],
                        rhs=h_fin[:, kh * B:(kh + 1) * B],
                        start=False, stop=(m == 1 and kh == 1),
                        skip_group_check=True,
                    )
            nc.scalar.activation(out=instrT[:], in_=pe2[:], func=AFT.Tanh)
            lstm_psum.__exit__(None, None, None)

            # ---- lin1 (c-chunk slice): core k computes f1[all r, c-chunk k]
            # for all 16 samples.  PSUM partitions = c_local; psum cols
            # (b, r) b-major so the evacuation ACT is contiguous on both
            # sides.  f1_sb cols = b*256 + r (b-major, fp8, 256*f1).
            f1_sb = f1pool.tile([P, B * R], fp8)
            a2a_in = [dpool.tile([N_CORES * P, BS * P], fp8,
                                 name=f"a2ai{h}") for h in range(2)]
            a2a_out = [dpool.tile([N_CORES * P, BS * P], fp8,
                                  name=f"a2ao{h}") for h in range(2)]
            f1_bv = f1_sb[:].rearrange("p (b r) -> p b r", b=B)
            f1q4 = f1_sb[:].rearrange("p (j b2 rg) -> p j b2 rg",
                                      j=N_CORES, b2=BS)

            def a2a_launch(h):
                # staging per (ring, b2): 3-dim APs (DMA limit)
                iv = a2a_in[h][:].rearrange("(j c) (b2 r) -> c j b2 r",
                                            j=N_CORES, b2=BS)
                for b2 in range(BS):
                    eng = nc.sync if b2 == 0 else nc.scalar
                    eng.dma_start(out=iv[:, :, b2],
                                  in_=f1q4[:, :, b2, h * P:(h + 1) * P])
                nc.gpsimd.collective_compute(
                    "AllToAll", AluOpType.bypass,
                    replica_groups=[list(range(N_CORES))],
                    ins=[a2a_in[h][:].opt()], outs=[a2a_out[h][:].opt()],
                )

            lin1_psum = tc.tile_pool(name="lpsum", bufs=4, space="PSUM")
            lpsum = lin1_psum.__enter__()
            for rt in range(8):
                wch = wchs[rt]
                pb = lpsum.tile([P, RS * B], f32, tag="lin1")
                pb_v = pb[:].rearrange("p (b r) -> p b r", b=B)
                nc.tensor.matmul(pb[:], lhsT=b1[:, rt * P:(rt + 1) * P],
                                 rhs=delta[:], start=True, stop=False,
                                 skip_group_check=True)
                for r in range(RS):
                    for kh in range(2):
                        nc.tensor.matmul(
                            pb_v[:, :, r],
                            lhsT=wch[:, (r * 2 + kh) * P:(r * 2 + kh + 1) * P],
                            rhs=instrT[:, kh * B:(kh + 1) * B],
                            start=False, stop=(r == RS - 1 and kh == 1),
                            skip_group_check=True,
                        )
                # f1q = prelu(psum * (SC_F1/SC_W1)); contiguous both sides
                nc.scalar.activation(
                    out=f1_bv[:, :, rt * RS:(rt + 1) * RS], in_=pb_v[:],
                    func=AFT.Prelu, alpha=0.01, scale=SC_F1 / SC_W1)
                if rt == 3:
                    # r-half 0 complete: trigger the first AllToAll so its
                    # wire time overlaps lin1's second half.
                    a2a_launch(0)
            lin1_psum.__exit__(None, None, None)
            a2a_launch(1)

            # ---- conv + fused BN-shift + channel max ----------------------
            # Per half m: gather shard kc (= c-chunk kc of my 2 samples,
            # 128 r) and immediately accumulate it into the 4 (b, n) PSUM
            # tiles; consumer DMAs pipeline with the matmuls.  BN shift
            # enters via the evacuation ACT bias (per-partition tvec);
            # ACT scale 1/SC_F1 unscales the fp8 filter quantization.
            vout = vpool.tile([P, BS * 2 * 2 * 16], f32)  # [(j,q),(b,n,m,blk)]
            NH = HW2 // 2  # 512
            inv_f1 = 1.0 / SC_F1
            conv_psum = tc.tile_pool(name="cpsum", bufs=8, space="PSUM")
            cpsum = conv_psum.__enter__()
            for m in range(2):
                # per-sample gather tiles (separate tiles: Tile tracks deps
                # at tile granularity, so sample 0's matmuls start as soon
                # as its own gather lands)
                ov4 = a2a_out[m][:].rearrange("(kc c) (b r) -> b c kc r",
                                              kc=N_CORES, b=BS)
                f1cb = []
                for b in range(BS):
                    fc = f1cpool.tile([P, 8 * P], fp8, tag=f"f1c{b}",
                                      name=f"f1c{m}{b}")
                    eng = nc.sync if b == 0 else nc.scalar
                    eng.dma_start(
                        out=fc[:].rearrange("p (kc r) -> p kc r", kc=8),
                        in_=ov4[b])
                    f1cb.append(fc[:].rearrange("p (kc r) -> p kc r", kc=8))
                # tile-major: finish one (b, n) tile's full kc contraction,
                # evacuate it immediately while the next tile's matmuls run
                for b in range(BS):
                    for n in range(2):
                        pc = cpsum.tile([P, NH], f32, tag="conv",
                                        name=f"pc{m}{b}{n}")
                        for kc in range(8):
                            nc.tensor.matmul(
                                pc[:],
                                lhsT=f1cb[b][:, kc],
                                rhs=fbs[b][:, kc * HW2 + n * NH:
                                           kc * HW2 + (n + 1) * NH],
                                start=(kc == 0), stop=(kc == 7),
                                skip_group_check=True,
                            )
                        cp = rmpool.tile([P, NH], f32, tag="convcp")
                        nc.scalar.activation(out=cp[:], in_=pc[:],
                                             func=AFT.Identity,
                                             scale=inv_f1,
                                             bias=tvec[:, m:m + 1])
                        col = ((b * 2 + n) * 2 + m) * 16
                        nc.vector.tensor_reduce(
                            out=vout[:, col:col + 16],
                            in_=cp[:].rearrange("p (blk q) -> p blk q", q=32),
                            axis=AX.X, op=AluOpType.max, apply_transpose=True)
                # store this half's columns as soon as its reduces land, so
                # the m=0 store overlaps conv m=1
                ov3 = out_d.rearrange("p (bn m blk) -> p bn m blk",
                                      m=2, blk=16)
                vv3 = vout[:].rearrange("p (bn m blk) -> p bn m blk",
                                        m=2, blk=16)
                nc.sync.dma_start(out=ov3[:, :, m], in_=vv3[:, :, m])
            conv_psum.__exit__(None, None, None)

    _split_excess_waits(nc)
    return nc


# ---------------------------------------------------------------------------
def _prep_inputs(feature, instruction_idx, instruction_length, emb_table,
                 w_ih, w_hh, b_ih, b_hh, e2d_w, e2d_b,
                 lin1_w, lin1_b, bn_gamma, bn_beta, bn_mean, bn_var):
    """Host-side layout/dtype prep. Returns (in_maps, slots, T0)."""
    f32 = np.float32

    def to_bf(x):
        return np.ascontiguousarray(x.astype(BF16))

    def to_f8(x, scale):
        return np.ascontiguousarray((x * scale).astype(FP8))

    feature = np.asarray(feature, f32)
    emb_table = np.asarray(emb_table, f32)
    idx = np.asarray(instruction_idx)
    lengths = np.asarray(instruction_length).astype(np.int64)
    slots = [int(max(l, 1) - 1) for l in lengths]

    # feature (b, c_in, kc, hw): per-partition data contiguous (16KB)
    feat = to_bf(feature.reshape(B, 8, P, HW2).transpose(0, 2, 1, 3))

    # embeds transposed: [p, (ke, t*b)]
    emb = emb_table[idx]                       # [B, S, E]
    embT = emb.transpose(2, 1, 0).reshape(2, P, S * B)
    embT = to_bf(embT.transpose(1, 0, 2).reshape(P, 2 * S * B))

    def wtiles_f32(w, kt, mt):
        # w: [out, in] -> lhsT tiles arr[p, (k, m, col)] with lhsT=w.T tile
        wt = np.asarray(w, f32).T  # [in, out]
        a = wt.reshape(kt, P, mt, P).transpose(1, 0, 2, 3)
        return a.reshape(P, kt * mt * P)

    # tanh(g) computed as 2*sigmoid(2g)-1: scale the g-gate rows (512:768)
    # by 2 so one big sigmoid covers all four gates.  The whole gate path
    # carries a 256x scale (fp8 W_hh) that the sigmoid's input scale undoes.
    gsc = np.ones((4 * HID, 1), f32)
    gsc[2 * HID:3 * HID] = 2.0
    wihT = to_bf(wtiles_f32(np.asarray(w_ih, f32) * gsc * SC_HH, 2, 8))
    whh8 = to_f8(wtiles_f32(np.asarray(w_hh, f32) * gsc, 2, 8), SC_HH)
    e2dT = to_bf(wtiles_f32(e2d_w, 2, 2))

    bg = ((np.asarray(b_ih, f32) + np.asarray(b_hh, f32)) * gsc[:, 0]
          * SC_HH).reshape(8, P).T.copy()
    e2db = np.zeros((P, 2), f32)  # biasf slots kept for layout stability
    # e2d bias as a rank-2 matmul: lhsT [2, 128] (bias rows), selector [2, 32]
    e2dsel = np.zeros((2, 2 * B), f32)
    e2dsel[0, 0:B] = 1.0
    e2dsel[1, B:2 * B] = 1.0
    e2dbias_row = np.concatenate(
        [np.asarray(e2d_b, f32).reshape(2, P), e2dsel], axis=1)  # [2, P+32]

    s = np.asarray(bn_gamma, f32) / np.sqrt(np.asarray(bn_var, f32) + BN_EPS)
    tsh = np.asarray(bn_beta, f32) - np.asarray(bn_mean, f32) * s
    T0 = float(tsh.max())

    w1s = np.asarray(lin1_w, f32).reshape(R, C, HID) * s[:, None, None]
    b1s = np.asarray(lin1_b, f32).reshape(R, C) * s[:, None] * SC_W1

    # delta b-major: delta[r', b*RS + r] = (r' == r)
    delta = np.tile(np.eye(RS, dtype=f32), (1, B))  # [32, 512]
    eye = np.eye(P, dtype=f32)

    def pad128(a):
        out = np.zeros((P, a.shape[1]), f32)
        out[:a.shape[0]] = a
        return out

    # biasf: gate biases (x256) | e2d bias | BN-shift per conv r-tile
    tvec = tsh.reshape(2, P).T.copy()
    biasf = np.concatenate([bg, e2db, tvec], axis=1).astype(f32)  # [128, 12]
    biasf = np.ascontiguousarray(biasf)

    in_maps = []
    for k in range(N_CORES):
        csl = slice(k * P, (k + 1) * P)
        wsl = w1s[:, csl]                       # [256, 128, 256] (r, c, h)
        # tiles (rt, r_local, kh): arr[p, ...] = W'T[kh*128+p, rt*32+rl, c]
        ws = wsl.transpose(2, 0, 1)             # [h, r, c]
        a = (ws.reshape(2, P, 8, RS, P)         # [kh, p, rt, rl, c]
             .transpose(1, 2, 3, 0, 4)          # [p, rt, rl, kh, c]
             .reshape(P, 8 * RS * 2 * P))
        # b1 inject tile per rt: [rl, (rt, c)]
        b1c = (b1s[:, csl].reshape(8, RS, P)    # [rt, rl, c]
               .transpose(1, 0, 2).reshape(RS, 8 * P))
        cpack = np.concatenate(
            [embT.astype(f32), wihT.astype(f32),
             e2dT.astype(f32), eye, pad128(b1c), pad128(delta),
             pad128(e2dbias_row)], axis=1)
        in_maps.append(dict(feat=feat[2 * k:2 * k + 2], cpack=to_bf(cpack),
                            whh8=whh8, biasf=biasf,
                            w1T=to_f8(a, SC_W1)))
    return in_maps, slots, T0


_cache = {}


def _run(inputs, trace=False):
    (in_maps, slots, T0) = _prep_inputs(
        inputs["feature"], inputs["instruction_idx"],
        inputs["instruction_length"], inputs["emb_table"],
        inputs["w_ih"], inputs["w_hh"], inputs["b_ih"], inputs["b_hh"],
        inputs["e2d_w"], inputs["e2d_b"], inputs["lin1_w"], inputs["lin1_b"],
        inputs["bn_gamma"], inputs["bn_beta"], inputs["bn_mean"],
        inputs["bn_var"])

    key = tuple(slots)
    if key not in _cache:
        _cache[key] = _build_program(slots)
    nc = _cache[key]

    kw = {}
    if trace:
        kw = dict(trace=True, trace_cores=list(range(N_CORES)))
    res = run_bass_kernel_spmd(nc, in_maps, list(range(N_CORES)), **kw)
    # per-core out: [128=(j,q), 128=(b,n,m,blk)]; sample = 2*core + b,
    # hw = n*512 + blk*32 + q, value = max over 32-row group j of r-tile m.
    parts = np.stack([np.asarray(res.results[i]["part_out"], np.float32)
                      for i in range(N_CORES)])      # [8, 128, 128]
    v = parts.reshape(N_CORES, 4, 32, BS, 2, 2, 16)  # [c, j, q, b, n, m, blk]
    v = v.max(axis=(1, 5))                           # [core, q, b, n, blk]
    v = v.transpose(0, 2, 3, 4, 1)                   # [core, b, n, blk, q]
    single = v.reshape(B, HW2)
    single = np.maximum(single, T0)
    out = np.clip(single, 0.0, 1.0).reshape(B, 32, 32).astype(np.float32)
    return out, res


def kernel(**inputs) -> np.ndarray:
    out, _ = _run(inputs, trace=False)
    return out


def kernel_traced(**inputs):
    out, res = _run(inputs, trace=True)
    return out, res


# revision 31
# speedup vs baseline: 1.0269x; 1.0269x over previous
"""Trainium2 Bass kernel for nn_Map_79748952752358 (dense_cnn).

Pipeline: LSTM encoder (40 steps) -> e2d projection -> big linear
(lin1: 256 -> 262144) -> per-sample dynamic 1x1 conv over feature
[1024, 32x32] -> BN(eval) -> channel-max -> clip.

Sharding (v3):
  - LSTM + e2d replicated on all 8 cores (serial recurrence, tiny state).
  - lin1 tensor-sharded over C: core k computes f1[all r, c-chunk k] for
    all 16 samples (W1 shard streams in fp8, 8.4MB/core, fully resident
    in SBUF before lin1 starts).
  - Two AllToAlls (one per r-half) redistribute filters so core k holds
    the full [256, 1024] filter block for ITS two samples; payload fp8.
  - conv batch-sharded: each core convolves its 2 samples with full
    R=256; BN shift enters via the evacuation ACT bias.
  - per-core output: channel-max partials [128, 128]; host combines the
    32-row groups, applies the BN-shift floor T0 and the clip.

Quantization (exact power-of-2 scale folds, verified vs fp32 reference
host-side: l2_rel ~5.3e-3 < 2e-2 gate):
  - W1 stored fp8e4 as 512*W1'; bias inject 512*b1; evacuation computes
    f1q = prelu(psum * 0.5) = 256*lrelu(W1'x+b1) stored fp8e4.
  - conv PSUM = 256*rele; evacuation ACT scale 1/256 + BN-shift bias.
  - W_hh stored fp8e4 as 256*w (g-rows pre-doubled for the
    tanh(g)=2*sig(2g)-1 trick); w_ih/bias scaled 256 in bf16/f32 so the
    gate PSUM is 256*gates; sigmoid ACT scale 1/256 unscales exactly.
"""

import numpy as np
import ml_dtypes

import concourse.bass as bass
import concourse.mybir as mybir
from concourse import tile
from concourse.tile import ScopedClock
from concourse.alu_op_type import AluOpType
from concourse.bass_utils import run_bass_kernel_spmd

BF16 = ml_dtypes.bfloat16
FP8 = ml_dtypes.float8_e4m3

B, S, V, E, HID = 16, 40, 1004, 256, 256
C, R, HW2 = 1024, 256, 1024
BN_EPS = 1e-5
N_CORES = 8
RS = R // N_CORES   # 32 r-rows per lin1 psum tile
BS = B // N_CORES   # 2 samples per core (conv shard)
P = 128

SC_HH = 256.0   # gate-path scale (w_ih, w_hh, gate bias, xg)
SC_W1 = 512.0   # W1 fp8 scale
SC_F1 = 256.0   # f1 fp8 scale

AFT = mybir.ActivationFunctionType
AX = mybir.AxisListType


# ---------------------------------------------------------------------------
# Tile tail-drain patch: this walrus build accepts fewer sem waits per
# TPB_CTRL instruction than Tile's exit drain accumulates; split them into
# single-wait SP nops.
_drain_patched = False


def _patch_tile_drain():
    global _drain_patched
    if _drain_patched:
        return
    _drain_patched = True

    def _patched(self, tick_clock, wait_clock):
        nc = self.nc
        probe = nc.sync.nop(nofuse=True, hint="drain_wait_split")
        wait_clock.add_sem_waits(
            probe.ins, ScopedClock({None: tick_clock.global_clock})
        )
        si = probe.ins.sync_info
        waits = list(si.on_wait or []) if si is not None else []
        if len(waits) > 1:
            si.on_wait = waits[:1]
            for w in waits[1:]:
                n = nc.sync.nop(nofuse=True, hint="drain_wait_split")
                nsi = n.ins.sync_info
                if nsi is None:
                    import bass_rust

                    n.ins.sync_info = bass_rust.SyncInfo(on_wait=[w], on_update=[])
                else:
                    nsi.on_wait = [w]
        nc.sync.drain()
        nc.all_engine_barrier()
        assert self.sems is not None
        popped = nc._tile_sem_poison_stack.pop()
        assert popped is self._sem_poison
        nc.clear_and_free_semaphores(list(self.sems.allocated().values()))
        nc.all_engine_barrier()

    tile.TileContext._drain_and_barrier = _patched


_ws_counter = [0]


def _split_excess_waits(nc, limit=1):
    """Walrus on this image rejects instructions with more than ~2 sem waits.
    Move excess waits onto same-engine EventSemaphore carriers inserted just
    before the offending instruction (same per-engine stream order, identical
    blocking semantics)."""
    import bass_rust

    for fn in nc.m.functions:
        for bb in fn.blocks:
            out = []
            for inst in bb.instructions:
                si = inst.sync_info
                waits = list(si.on_wait or []) if si is not None else []
                if len(waits) > limit:
                    for w in waits[:-limit]:
                        _ws_counter[0] += 1
                        carrier = mybir.InstEventSemaphore(
                            name=f"I-waitsplit-{_ws_counter[0]}",
                            opcode="EventSemaphore",
                            engine=inst.engine,
                            sync_info=bass_rust.SyncInfo(
                                on_wait=[w], on_update=[]),
                        )
                        out.append(carrier)
                    si.on_wait = waits[-limit:]
                out.append(inst)
            bb.instructions = out


# ---------------------------------------------------------------------------
def _build_program(slots):
    """Build the SPMD Bass program. `slots[b]` = length[b]-1, the LSTM step
    whose hidden state is each sample's final state (compile-time constants).
    """
    _patch_tile_drain()
    nc = bass.Bass("TRN2", target_bir_lowering=False, debug=False,
                   num_devices=N_CORES)
    dt = mybir.dt
    f32, bf16, fp8 = dt.float32, dt.bfloat16, dt.float8e4

    def din(name, shape, d=bf16):
        return nc.dram_tensor(name, shape, d, kind="ExternalInput").ap()

    # feature slice for this core's 2 samples: (b, c_in_chunk, kc, hw)
    feat_d = din("feat", [BS, P, 8, HW2])
    # packed bf16 constant block: embT | wihT | e2dT | eye | b1 | delta |
    # e2dbias [2,128] | e2dsel [2,32]
    PK = [2 * S * B, 16 * P, 4 * P, P, 8 * P, RS * B, P + 2 * B]
    pack_d = din("cpack", [P, sum(PK)])
    whh_d = din("whh8", [P, 16 * P], fp8)
    biasf_d = din("biasf", [P, 12], f32)   # biasg(8)*256 | e2db(2) | tvec(2)
    w1_d = din("w1T", [P, 8 * RS * 2 * P], fp8)   # tiles (rt, r, kh)

    out_d = nc.dram_tensor("part_out", [P, BS * 2 * 2 * 16], f32,
                           kind="ExternalOutput").ap()

    with tile.TileContext(nc) as tc:
        with (
            tc.tile_pool(name="const", bufs=1) as cpool,
            tc.tile_pool(name="xg", bufs=1) as xgpool,
            tc.tile_pool(name="hist", bufs=1) as hpool,
            tc.tile_pool(name="gs", bufs=2) as gspool,
            tc.tile_pool(name="cell", bufs=1) as cellpool,
            tc.tile_pool(name="tmp", bufs=4) as tmppool,
            tc.tile_pool(name="w1c", bufs=8) as w1pool,
            tc.tile_pool(name="f1", bufs=1) as f1pool,
            tc.tile_pool(name="feat", bufs=2) as fpool,
            tc.tile_pool(name="f1c", bufs=2) as f1cpool,
            tc.tile_pool(name="rmax", bufs=4) as rmpool,
            tc.tile_pool(name="vout", bufs=1) as vpool,
            tc.tile_pool(name="a2a", bufs=1, space="DRAM") as dpool,
        ):
            pack = cpool.tile([P, sum(PK)], bf16, tag="cpack")
            # split the pack load: embT+wih first so stage A starts early
            cut = PK[0] + PK[1]
            nc.sync.dma_start(out=pack[:, 0:cut], in_=pack_d[:, 0:cut])
            biasf = cpool.tile([P, 12], f32, tag="biasf")
            nc.sync.dma_start(out=biasf[:], in_=biasf_d)
            whh = cpool.tile([P, 16 * P], fp8, tag="whh8")
            nc.sync.dma_start(out=whh[:], in_=whh_d)
            nc.sync.dma_start(out=pack[:, cut:], in_=pack_d[:, cut:])

            # prepull the sigmoid/tanh ACT table set with a dummy activation
            # so the one-time ~1.3us table load overlaps the pack DMA
            # instead of delaying LSTM step 0.
            warm = cpool.tile([1, 1], f32, tag="warm")
            nc.scalar.activation(out=warm[:], in_=biasf[0:1, 0:1],
                                 func=AFT.Sigmoid)

            # early cross-core barrier: aligns all cores while the LSTM
            # runs (gpsimd/ncfw idle), so the real AllToAll's entry skew
            # shrinks.  Costs nothing on the critical path.
            bar_sb = cpool.tile([1, 8], f32, tag="bar")
            nc.vector.memset(bar_sb[:], 1.0)
            bar_in = dpool.tile([1, 8], f32, name="barin")
            bar_out = dpool.tile([1, 8], f32, name="barout")
            nc.gpsimd.dma_start(out=bar_in[:], in_=bar_sb[:])
            nc.gpsimd.collective_compute(
                "AllReduce", AluOpType.add,
                replica_groups=[list(range(N_CORES))],
                ins=[bar_in[:].opt()], outs=[bar_out[:].opt()],
            )

            off = np.cumsum([0] + PK)
            embT = pack[:, off[0]:off[1]]
            wih = pack[:, off[1]:off[2]]
            e2dT = pack[:, off[2]:off[3]]
            eye = pack[:, off[3]:off[4]]
            b1 = pack[0:RS, off[4]:off[5]]
            delta = pack[0:RS, off[5]:off[6]]
            e2dbias = pack[0:2, off[6]:off[6] + P]        # [2, 128]
            e2dsel = pack[0:2, off[6] + P:off[7]]         # [2, 32]
            biasg = biasf[:, 0:8]
            tvec = biasf[:, 10:12]                # BN shift per conv r-tile

            # ---- prefetched loads: all 8 w1 chunks (fp8), feature --------
            CW = RS * 2 * P  # w1 chunk cols per ct

            def w1_fetch(ct):
                wch = w1pool.tile([P, CW], fp8, tag="w1c", name=f"wch{ct}")
                nc.sync.dma_start(out=wch[:], in_=w1_d[:, ct * CW:(ct + 1) * CW])
                return wch

            wchs = []
            for ct in range(8):
                wchs.append(w1_fetch(ct))
                if ct == 3:
                    # feature slice DMA sits between w1 chunk 3 and 4 so it
                    # is in flight well before the conv needs it.
                    fbs = []
                    for b in range(BS):
                        fb = fpool.tile([P, 8 * HW2], bf16, tag="feat",
                                        name=f"fb{b}")
                        nc.sync.dma_start(
                            out=fb[:].rearrange("p (kc hw) -> p kc hw", kc=8),
                            in_=feat_d[b])
                        fbs.append(fb)

            # ---- Stage A: xg = 256*(w_ih @ x_t + bg) for all steps -------
            # n=0 (steps 0-19) runs before the LSTM; n=1 (steps 20-39) is
            # emitted inside LSTM steps 0-7 to fill engine idle slots.
            xg_s = xgpool.tile([P, 8 * S * B], bf16)
            NCH = 320  # psum N-chunk: 20 steps x 16
            lstm_psum = tc.tile_pool(name="gpsum", bufs=2, space="PSUM")
            gpsum = lstm_psum.__enter__()
            xpsum_cm = tc.tile_pool(name="xpsum", bufs=2, space="PSUM")
            xpsum = xpsum_cm.__enter__()

            def stage_a(n, m):
                ps = xpsum.tile([P, NCH], f32, tag="xg")
                for ke in range(2):
                    nc.tensor.matmul(
                        ps[:],
                        lhsT=wih[:, (ke * 8 + m) * P:(ke * 8 + m + 1) * P],
                        rhs=embT[:, ke * S * B + n * NCH: ke * S * B + (n + 1) * NCH],
                        start=(ke == 0), stop=(ke == 1),
                    )
                dst = xg_s[:, m * S * B + n * NCH: m * S * B + (n + 1) * NCH]
                # n=1 evacs (emitted inside LSTM steps) go on the DVE: the
                # ACT queue is the LSTM chain's bottleneck engine there.
                if n == 0 and m % 2 == 0:
                    nc.scalar.activation(out=dst, in_=ps[:], func=AFT.Identity,
                                         bias=biasg[:, m:m + 1])
                else:
                    nc.vector.tensor_scalar_add(dst, ps[:], biasg[:, m:m + 1])

            for m in range(8):
                stage_a(0, m)

            # ---- Stage B: LSTM recurrence (layout: gate-dim on partitions) --
            # gates psum split: ifg (cols 0:96) and o (32) so the sigmoid of
            # i,f,g starts after 12 of the 16 whh matmuls and sig_o overlaps
            # the DVE c-update chain.
            S_eff = max(slots) + 1
            hist = hpool.tile([P, S * 2 * B], bf16)   # (t, kh, b)
            c_s = cellpool.tile([P, 2 * B], f32)      # (kh, b)
            xg_r = xg_s[:].rearrange("p (m t b) -> p m t b", m=8, t=S)
            # final-h capture target (filled inside the loop at t==slots[b])
            h_fin = cellpool.tile([P, 2 * B], bf16, tag="hfin")  # (kh, b)
            hf_r = h_fin[:].rearrange("p (k b) -> p b k", k=2)
            inv_hh = 1.0 / SC_HH

            def whh_mm(dst, dcol, m, kh, t, last):
                nc.tensor.matmul(
                    dst[:, dcol * B:(dcol + 1) * B],
                    lhsT=whh[:, (kh * 8 + m) * P:(kh * 8 + m + 1) * P],
                    rhs=hist[:, (t - 1) * 2 * B + kh * B:
                             (t - 1) * 2 * B + (kh + 1) * B],
                    start=False, stop=last, skip_group_check=True,
                )

            for t in range(S_eff):
                # psum/ACT split (g), (if), (o): sig_g lands first so the
                # tg tensor_scalar overlaps sig_if; sig_o overlaps the DVE
                # c-chain.  Gate tiles: i=m0-1, f=m2-3, g=m4-5, o=m6-7.
                # separate psum TILES per gate group: Tile tracks psum reads
                # at tile granularity, so sig_g can fire right after the 4
                # g-gate matmuls instead of all 16.  Separate tiles also sit
                # in separate banks, keeping one start=True per bank.
                gg_t = gpsum.tile([P, 32], f32, tag="gg")
                gif_t = gpsum.tile([P, 64], f32, tag="gif")
                go_t = gpsum.tile([P, 32], f32, tag="go")
                gg, gif, go = gg_t[:], gif_t[:], go_t[:]
                nc.tensor.matmul(gg, lhsT=eye[:], rhs=xg_r[:, 4:6, t, :],
                                 start=True, stop=(t == 0),
                                 skip_group_check=True)
                nc.tensor.matmul(gif, lhsT=eye[:], rhs=xg_r[:, 0:4, t, :],
                                 start=True, stop=(t == 0),
                                 skip_group_check=True)
                nc.tensor.matmul(go, lhsT=eye[:], rhs=xg_r[:, 6:8, t, :],
                                 start=True, stop=(t == 0),
                                 skip_group_check=True)
                if t > 0:
                    for m in (4, 5):
                        for kh in range(2):
                            whh_mm(gg, m - 4, m, kh, t, m == 5 and kh == 1)
                    for m in range(4):
                        for kh in range(2):
                            whh_mm(gif, m, m, kh, t, m == 3 and kh == 1)
                    for m in (6, 7):
                        for kh in range(2):
                            whh_mm(go, m - 6, m, kh, t, m == 7 and kh == 1)
                gsg = gspool.tile([P, 32], f32, tag="gsg")
                gsif = gspool.tile([P, 64], f32, tag="gsif")
                gso = gspool.tile([P, 32], f32, tag="gso")
                # tanh(g)=2*sig(2g)-1 (g-rows pre-scaled by 2 on host)
                nc.scalar.activation(out=gsg[:], in_=gg, func=AFT.Sigmoid,
                                     scale=inv_hh)
                nc.scalar.activation(out=gsif[:], in_=gif, func=AFT.Sigmoid,
                                     scale=inv_hh)
                nc.scalar.activation(out=gso[:], in_=go, func=AFT.Sigmoid,
                                     scale=inv_hh)
                # half-scale cell state: c' = c/2, so i*tanh(g) becomes one
                # fused op (sig_g - 0.5)*sig_i; tanh reads with scale=2.
                t1 = tmppool.tile([P, 2 * B], f32, tag="t1")
                if t == 0:
                    nc.vector.scalar_tensor_tensor(
                        t1[:], gsg[:], 0.5, gsif[:, 0:32],
                        AluOpType.subtract, AluOpType.mult)
                    nc.vector.tensor_copy(c_s[:], t1[:])
                else:
                    t2 = tmppool.tile([P, 2 * B], f32, tag="t2")
                    nc.vector.tensor_tensor(t2[:], gsif[:, 32:64], c_s[:],
                                            AluOpType.mult)
                    nc.vector.scalar_tensor_tensor(
                        t1[:], gsg[:], 0.5, gsif[:, 0:32],
                        AluOpType.subtract, AluOpType.mult)
                    nc.vector.tensor_tensor(c_s[:], t1[:], t2[:], AluOpType.add)
                th = tmppool.tile([P, 2 * B], bf16, tag="th")
                nc.scalar.activation(out=th[:], in_=c_s[:], func=AFT.Tanh,
                                     scale=2.0)
                nc.vector.tensor_tensor(
                    hist[:, t * 2 * B:(t + 1) * 2 * B],
                    gso[:], th[:], AluOpType.mult)
                # capture final h for samples ending at this step (hides in
                # the per-step DVE slack)
                src = hist[:, t * 2 * B:(t + 1) * 2 * B]
                for b in range(B):
                    if slots[b] == t:
                        nc.vector.tensor_copy(
                            hf_r[:, b],
                            src.rearrange("p (k b) -> p b k", k=2)[:, b])
                # stage A n=1: fills the PE/ACT/DVE idle slots of the first
                # 8 steps (needed from step 20 onward only)
                if t < 8:
                    stage_a(1, t)
            xpsum_cm.__exit__(None, None, None)

            # ---- e2d projection: instrT = tanh(e2d_w @ h + b) -------------
            # one psum group; bias injected via rank-1 matmuls so a single
            # tanh ACT covers both m-tiles.
            instrT = cellpool.tile([P, 2 * B], bf16, tag="instrT")  # (kh, b)
            pe2 = gpsum.tile([P, 2 * B], f32, tag="gg")  # reuse gate bank
            # rank-2 bias inject: one start=True matmul covers both m-tiles
            nc.tensor.matmul(pe2[:], lhsT=e2dbias[0:2, 0:P],
                             rhs=e2dsel[0:2, :],
                             start=True, stop=False, skip_group_check=True)
            for m in range(2):
                for kh in range(2):
                    nc.tensor.matmul(
                        pe2[:, m * B:(m + 1) * B],
                        lhsT=e2dT[:, (kh * 2 + m) * P:(kh * 2 + m + 1) * P],
                        rhs=h_fin[:, kh * B:(kh + 1) * B],
                        start=False, stop=(m == 1 and kh == 1),
                        skip_group_check=True,
                    )
            nc.scalar.activation(out=instrT[:], in_=pe2[:], func=AFT.Tanh)
            lstm_psum.__exit__(None, None, None)

            # ---- lin1 (c-chunk slice): core k computes f1[all r, c-chunk k]
            # for all 16 samples.  PSUM partitions = c_local; psum cols
            # (b, r) b-major so the evacuation ACT is contiguous on both
            # sides.  f1_sb cols = b*256 + r (b-major, fp8, 256*f1).
            f1_sb = f1pool.tile([P, B * R], fp8)
            a2a_in = [dpool.tile([N_CORES * P, BS * P], fp8,
                                 name=f"a2ai{h}") for h in range(2)]
            a2a_out = [dpool.tile([N_CORES * P, BS * P], fp8,
                                  name=f"a2ao{h}") for h in range(2)]
            f1_bv = f1_sb[:].rearrange("p (b r) -> p b r", b=B)
            f1q4 = f1_sb[:].rearrange("p (j b2 rg) -> p j b2 rg",
                                      j=N_CORES, b2=BS)

            def a2a_launch(h):
                # staging per (ring, b2): 3-dim APs (DMA limit)
                iv = a2a_in[h][:].rearrange("(j c) (b2 r) -> c j b2 r",
                                            j=N_CORES, b2=BS)
                for b2 in range(BS):
                    eng = nc.sync if b2 == 0 else nc.scalar
                    eng.dma_start(out=iv[:, :, b2],
                                  in_=f1q4[:, :, b2, h * P:(h + 1) * P])
                nc.gpsimd.collective_compute(
                    "AllToAll", AluOpType.bypass,
                    replica_groups=[list(range(N_CORES))],
                    ins=[a2a_in[h][:].opt()], outs=[a2a_out[h][:].opt()],
                )

            lin1_psum = tc.tile_pool(name="lpsum", bufs=4, space="PSUM")
            lpsum = lin1_psum.__enter__()
            for rt in range(8):
                wch = wchs[rt]
                pb = lpsum.tile([P, RS * B], f32, tag="lin1")
                pb_v = pb[:].rearrange("p (b r) -> p b r", b=B)
                nc.tensor.matmul(pb[:], lhsT=b1[:, rt * P:(rt + 1) * P],
                                 rhs=delta[:], start=True, stop=False,
                                 skip_group_check=True)
                for r in range(RS):
                    for kh in range(2):
                        nc.tensor.matmul(
                            pb_v[:, :, r],
                            lhsT=wch[:, (r * 2 + kh) * P:(r * 2 + kh + 1) * P],
                            rhs=instrT[:, kh * B:(kh + 1) * B],
                            start=False, stop=(r == RS - 1 and kh == 1),
                            skip_group_check=True,
                        )
                # f1q = prelu(psum * (SC_F1/SC_W1)); contiguous both sides
                nc.scalar.activation(
                    out=f1_bv[:, :, rt * RS:(rt + 1) * RS], in_=pb_v[:],
                    func=AFT.Prelu, alpha=0.01, scale=SC_F1 / SC_W1)
                if rt == 3:
                    # r-half 0 complete: trigger the first AllToAll so its
                    # wire time overlaps lin1's second half.
                    a2a_launch(0)
            lin1_psum.__exit__(None, None, None)
            a2a_launch(1)

            # ---- conv + fused BN-shift + channel max ----------------------
            # Per half m: gather shard kc (= c-chunk kc of my 2 samples,
            # 128 r) and immediately accumulate it into the 4 (b, n) PSUM
            # tiles; consumer DMAs pipeline with the matmuls.  BN shift
            # enters via the evacuation ACT bias (per-partition tvec);
            # ACT scale 1/SC_F1 unscales the fp8 filter quantization.
            vout = vpool.tile([P, BS * 2 * 2 * 16], f32)  # [(j,q),(b,n,m,blk)]
            NH = HW2 // 2  # 512
            inv_f1 = 1.0 / SC_F1
            conv_psum = tc.tile_pool(name="cpsum", bufs=8, space="PSUM")
            cpsum = conv_psum.__enter__()
            for m in range(2):
                # per-sample gather tiles (separate tiles: Tile tracks deps
                # at tile granularity, so sample 0's matmuls start as soon
                # as its own gather lands)
                ov4 = a2a_out[m][:].rearrange("(kc c) (b r) -> b c kc r",
                                              kc=N_CORES, b=BS)
                f1cb = []
                for b in range(BS):
                    fc = f1cpool.tile([P, 8 * P], fp8, tag=f"f1c{b}",
                                      name=f"f1c{m}{b}")
                    eng = nc.sync if b == 0 else nc.scalar
                    eng.dma_start(
                        out=fc[:].rearrange("p (kc r) -> p kc r", kc=8),
                        in_=ov4[b])
                    f1cb.append(fc[:].rearrange("p (kc r) -> p kc r", kc=8))
                # tile-major: finish one (b, n) tile's full kc contraction,
                # evacuate it immediately while the next tile's matmuls run
                for b in range(BS):
                    for n in range(2):
                        pc = cpsum.tile([P, NH], f32, tag="conv",
                                        name=f"pc{m}{b}{n}")
                        for kc in range(8):
                            nc.tensor.matmul(
                                pc[:],
                                lhsT=f1cb[b][:, kc],
                                rhs=fbs[b][:, kc * HW2 + n * NH:
                                           kc * HW2 + (n + 1) * NH],
                                start=(kc == 0), stop=(kc == 7),
                                skip_group_check=True,
                            )
                        cp = rmpool.tile([P, NH], f32, tag="convcp")
                        nc.scalar.activation(out=cp[:], in_=pc[:],
                                             func=AFT.Identity,
                                             scale=inv_f1,
                                             bias=tvec[:, m:m + 1])
                        col = ((b * 2 + n) * 2 + m) * 16
                        nc.vector.tensor_reduce(
                            out=vout[:, col:col + 16],
                            in_=cp[:].rearrange("p (blk q) -> p blk q", q=32),
                            axis=AX.X, op=AluOpType.max, apply_transpose=True)
            conv_psum.__exit__(None, None, None)

            # contiguous store; host decodes the (j,q),(b,n,blk) layout
            nc.sync.dma_start(out=out_d, in_=vout[:])

    _split_excess_waits(nc)
    return nc


# ---------------------------------------------------------------------------
def _prep_inputs(feature, instruction_idx, instruction_length, emb_table,
                 w_ih, w_hh, b_ih, b_hh, e2d_w, e2d_b,
                 lin1_w, lin1_b, bn_gamma, bn_beta, bn_mean, bn_var):
    """Host-side layout/dtype prep. Returns (in_maps, slots, T0)."""
    f32 = np.float32

    def to_bf(x):
        return np.ascontiguousarray(x.astype(BF16))

    def to_f8(x, scale):
        return np.ascontiguousarray((x * scale).astype(FP8))

    feature = np.asarray(feature, f32)
    emb_table = np.asarray(emb_table, f32)
    idx = np.asarray(instruction_idx)
    lengths = np.asarray(instruction_length).astype(np.int64)
    slots = [int(max(l, 1) - 1) for l in lengths]

    # feature (b, c_in, kc, hw): per-partition data contiguous (16KB)
    feat = to_bf(feature.reshape(B, 8, P, HW2).transpose(0, 2, 1, 3))

    # embeds transposed: [p, (ke, t*b)]
    emb = emb_table[idx]                       # [B, S, E]
    embT = emb.transpose(2, 1, 0).reshape(2, P, S * B)
    embT = to_bf(embT.transpose(1, 0, 2).reshape(P, 2 * S * B))

    def wtiles_f32(w, kt, mt):
        # w: [out, in] -> lhsT tiles arr[p, (k, m, col)] with lhsT=w.T tile
        wt = np.asarray(w, f32).T  # [in, out]
        a = wt.reshape(kt, P, mt, P).transpose(1, 0, 2, 3)
        return a.reshape(P, kt * mt * P)

    # tanh(g) computed as 2*sigmoid(2g)-1: scale the g-gate rows (512:768)
    # by 2 so one big sigmoid covers all four gates.  The whole gate path
    # carries a 256x scale (fp8 W_hh) that the sigmoid's input scale undoes.
    gsc = np.ones((4 * HID, 1), f32)
    gsc[2 * HID:3 * HID] = 2.0
    wihT = to_bf(wtiles_f32(np.asarray(w_ih, f32) * gsc * SC_HH, 2, 8))
    whh8 = to_f8(wtiles_f32(np.asarray(w_hh, f32) * gsc, 2, 8), SC_HH)
    e2dT = to_bf(wtiles_f32(e2d_w, 2, 2))

    bg = ((np.asarray(b_ih, f32) + np.asarray(b_hh, f32)) * gsc[:, 0]
          * SC_HH).reshape(8, P).T.copy()
    e2db = np.zeros((P, 2), f32)  # biasf slots kept for layout stability
    # e2d bias as a rank-2 matmul: lhsT [2, 128] (bias rows), selector [2, 32]
    e2dsel = np.zeros((2, 2 * B), f32)
    e2dsel[0, 0:B] = 1.0
    e2dsel[1, B:2 * B] = 1.0
    e2dbias_row = np.concatenate(
        [np.asarray(e2d_b, f32).reshape(2, P), e2dsel], axis=1)  # [2, P+32]

    s = np.asarray(bn_gamma, f32) / np.sqrt(np.asarray(bn_var, f32) + BN_EPS)
    tsh = np.asarray(bn_beta, f32) - np.asarray(bn_mean, f32) * s
    T0 = float(tsh.max())

    w1s = np.asarray(lin1_w, f32).reshape(R, C, HID) * s[:, None, None]
    b1s = np.asarray(lin1_b, f32).reshape(R, C) * s[:, None] * SC_W1

    # delta b-major: delta[r', b*RS + r] = (r' == r)
    delta = np.tile(np.eye(RS, dtype=f32), (1, B))  # [32, 512]
    eye = np.eye(P, dtype=f32)

    def pad128(a):
        out = np.zeros((P, a.shape[1]), f32)
        out[:a.shape[0]] = a
        return out

    # biasf: gate biases (x256) | e2d bias | BN-shift per conv r-tile
    tvec = tsh.reshape(2, P).T.copy()
    biasf = np.concatenate([bg, e2db, tvec], axis=1).astype(f32)  # [128, 12]
    biasf = np.ascontiguousarray(biasf)

    in_maps = []
    for k in range(N_CORES):
        csl = slice(k * P, (k + 1) * P)
        wsl = w1s[:, csl]                       # [256, 128, 256] (r, c, h)
        # tiles (rt, r_local, kh): arr[p, ...] = W'T[kh*128+p, rt*32+rl, c]
        ws = wsl.transpose(2, 0, 1)             # [h, r, c]
        a = (ws.reshape(2, P, 8, RS, P)         # [kh, p, rt, rl, c]
             .transpose(1, 2, 3, 0, 4)          # [p, rt, rl, kh, c]
             .reshape(P, 8 * RS * 2 * P))
        # b1 inject tile per rt: [rl, (rt, c)]
        b1c = (b1s[:, csl].reshape(8, RS, P)    # [rt, rl, c]
               .transpose(1, 0, 2).reshape(RS, 8 * P))
        cpack = np.concatenate(
            [embT.astype(f32), wihT.astype(f32),
             e2dT.astype(f32), eye, pad128(b1c), pad128(delta),
             pad128(e2dbias_row)], axis=1)
        in_maps.append(dict(feat=feat[2 * k:2 * k + 2], cpack=to_bf(cpack),
                            whh8=whh8, biasf=biasf,
                            w1T=to_f8(a, SC_W1)))
    return in_maps, slots, T0


_cache = {}


def _run(inputs, trace=False):
    (in_maps, slots, T0) = _prep_inputs(
        inputs["feature"], inputs["instruction_idx"],
        inputs["instruction_length"], inputs["emb_table"],
        inputs["w_ih"], inputs["w_hh"], inputs["b_ih"], inputs["b_hh"],
        inputs["e2d_w"], inputs["e2d_b"], inputs["lin1_w"], inputs["lin1_b"],
        inputs["bn_gamma"], inputs["bn_beta"], inputs["bn_mean"],
        inputs["bn_var"])

    key = tuple(slots)
    if key not in _cache:
        _cache[key] = _build_program(slots)
    nc = _cache[key]

    kw = {}
    if trace:
        kw = dict(trace=True, trace_cores=list(range(N_CORES)))
    res = run_bass_kernel_spmd(nc, in_maps, list(range(N_CORES)), **kw)
    # per-core out: [128=(j,q), 128=(b,n,m,blk)]; sample = 2*core + b,
    # hw = n*512 + blk*32 + q, value = max over 32-row group j of r-tile m.
    parts = np.stack([np.asarray(res.results[i]["part_out"], np.float32)
                      for i in range(N_CORES)])      # [8, 128, 128]
    v = parts.reshape(N_CORES, 4, 32, BS, 2, 2, 16)  # [c, j, q, b, n, m, blk]
    v = v.max(axis=(1, 5))                           # [core, q, b, n, blk]
    v = v.transpose(0, 2, 3, 4, 1)                   # [core, b, n, blk, q]
    single = v.reshape(B, HW2)
    single = np.maximum(single, T0)
    out = np.clip(single, 0.0, 1.0).reshape(B, 32, 32).astype(np.float32)
    return out, res


def kernel(**inputs) -> np.ndarray:
    out, _ = _run(inputs, trace=False)
    return out


def kernel_traced(**inputs):
    out, res = _run(inputs, trace=True)
    return out, res
